# revision 14
# baseline (speedup 1.0000x reference)
"""Multi-head causal self-attention (B=2, S=2048, D=1024, H=16) on 8 trn2 cores.

Sharding: core c handles batch b = c // 4 and head-group g = c % 4 (4 heads,
columns 256*g .. 256*g+255 of Wq/Wk/Wv, rows of Wo). Each core computes a
partial [S, D] output (its heads' contribution through Wo); the host sums the
4 partials per batch and adds bo.

Per-core layout strategy (all tensors kept "transposed" so no on-chip
transposes are ever needed):
  X^T  [D, S]   passed from host pre-transposed/tiled
  Q^T: lhsT = Wq_shard [D, 256] (natural), rhs = X^T                 -> [256, S]
  K^T  same                                                          -> [256, S]
  V    natural: lhsT = X^T, rhs = Wv_shard                           -> [S, 256]
  S^T[k,q] per head: lhsT = K^T head slice [64, kblk], rhs = Q^T head slice.
        The two heads of a partition half run as a row-group pair
        (tile_position (0,0)/(64,0) via base_partition) so both K=64 matmuls
        occupy the PE array concurrently.
  P^T = exp(S^T / 8)  (no max-subtraction needed: logits are O(3) here),
        causality: blocks above the diagonal are skipped entirely,
        diagonal blocks zeroed with affine_select after exp.
  O^T[65, S] per head: lhsT = [V_head | ones] [kblk, 65], rhs = P^T
        (row 64 = softmax denominators, for free).
  normalize O^T rows by broadcast reciprocal of row 64 (per q-chunk,
        pipelined into the attention loop).
  partial^T = lhsT Wo_shard [256, D], rhs = O_cat^T [256, S], emitted
        one q-chunk behind attention so the PE never waits on it.
"""

import sys

sys.path.insert(0, "/opt/trn_rl_repo")

import numpy as np
import ml_dtypes

import concourse.bass as bass  # noqa: F401
import concourse.mybir as mybir
import concourse.tile as tile
from concourse import bacc
from concourse.bass_utils import run_bass_kernel_spmd

P = 128
B, S, D, H = 2, 2048, 1024, 16
HS = 64          # head size
NH = 4           # heads per core
DC = NH * HS     # 256 projected cols per core
KO = D // P      # 8 contraction blocks for projections
QC = 512         # q chunk (matmul moving free dim)
NQ = S // QC     # 4
NKB = S // P     # 16 k blocks
NCORES = 8

F32 = mybir.dt.float32
EXP = mybir.ActivationFunctionType.Exp


def build_program(mm_dt=mybir.dt.bfloat16):
    """Build the single-core Bass/Tile program (same program on all 8 cores)."""
    nc = bacc.Bacc("TRN2", target_bir_lowering=False, debug=False,
                   num_devices=NCORES)

    xt_d = nc.dram_tensor("xt", [P, KO, S], mm_dt, kind="ExternalInput")
    wq_d = nc.dram_tensor("wq", [P, KO, DC], mm_dt, kind="ExternalInput")
    wk_d = nc.dram_tensor("wk", [P, KO, DC], mm_dt, kind="ExternalInput")
    wv_d = nc.dram_tensor("wv", [P, KO, DC], mm_dt, kind="ExternalInput")
    wo_d = nc.dram_tensor("wo", [P, 2, D], mm_dt, kind="ExternalInput")
    bq_d = nc.dram_tensor("bq", [P, 2], F32, kind="ExternalInput")
    bk_d = nc.dram_tensor("bk", [P, 2], F32, kind="ExternalInput")
    bv_d = nc.dram_tensor("bvb", [P, DC], F32, kind="ExternalInput")
    out_d = nc.dram_tensor("out", [P, KO, S], mm_dt, kind="ExternalOutput")

    with tile.TileContext(nc) as tc, \
            tc.tile_pool(name="persist", bufs=1) as pp:
        # ---- persistent SBUF tensors (one slot per tag) ----
        xtk = [pp.tile([P, S], mm_dt, tag=f"xt{ko}", name=f"xt{ko}_sb")
               for ko in range(KO)]
        wq = pp.tile([P, KO, DC], mm_dt, tag="wq", name="wq_sb")
        wk = pp.tile([P, KO, DC], mm_dt, tag="wk", name="wk_sb")
        wv = pp.tile([P, KO, DC], mm_dt, tag="wv", name="wv_sb")
        wo = pp.tile([P, 2, D], mm_dt, tag="wo", name="wo_sb")
        bq_sb = pp.tile([P, 2], F32, tag="bq", name="bq_sb")
        bk_sb = pp.tile([P, 2], F32, tag="bk", name="bk_sb")
        bv_sb = pp.tile([P, DC], F32, tag="bv", name="bv_sb")
        qt = pp.tile([P, 2, S], mm_dt, tag="qt", name="qt_sb")  # head h rows 64h
        kt = pp.tile([P, 2, S], mm_dt, tag="kt", name="kt_sb")
        # V (+ ones column) per k block, fine-grained so attention can start
        # while the V projection is still running
        vx = [pp.tile([P, NH, HS + 1], mm_dt, tag=f"vx{kb}", name=f"vx{kb}_sb")
              for kb in range(NKB)]
        # normalized O^T per q-chunk, fine-grained for the pipelined Wo
        oc = [pp.tile([P, 2, QC], mm_dt, tag=f"oc{j}", name=f"oc{j}_sb")
              for j in range(NQ)]

        # ---- load inputs, spread across engine DMA queues so the
        # startup load isn't serialized through one HW-DGE queue ----
        engs = [nc.sync, nc.gpsimd, nc.scalar]
        nc.sync.dma_start(wq[:], wq_d[:])
        nc.gpsimd.dma_start(wk[:], wk_d[:])
        nc.sync.dma_start(bq_sb[:], bq_d[:])
        nc.sync.dma_start(bk_sb[:], bk_d[:])
        for ko in range(KO):
            engs[ko % 3].dma_start(xtk[ko][:], xt_d[:, ko, :])
        nc.scalar.dma_start(wv[:], wv_d[:])
        nc.scalar.dma_start(bv_sb[:], bv_d[:])
        nc.gpsimd.dma_start(wo[:], wo_d[:])
        for kb in range(NKB):
            nc.gpsimd.memset(vx[kb][:, :, HS:HS + 1], 1.0)

        with (
            tc.tile_pool(name="ps_s", bufs=2, space="PSUM") as ps_s,
            tc.tile_pool(name="ps_o", bufs=2, space="PSUM") as ps_o,
            tc.tile_pool(name="ps_w", bufs=2, space="PSUM") as ps_w,
            tc.tile_pool(name="ptp", bufs=18) as ptp,
            tc.tile_pool(name="otp", bufs=6) as otp,
            tc.tile_pool(name="stg", bufs=2) as stg,
        ):
            # ---- Q^T / K^T projections: [256, S] as 2 partition halves ----
            for w_sb, b_sb, dst in ((wq, bq_sb, qt), (wk, bk_sb, kt)):
                for half in range(2):
                    for jq in range(NQ):
                        ps = ps_w.tile([P, QC], F32, tag="w", name="ps_qk")
                        for ko in range(KO):
                            nc.tensor.matmul(
                                ps[:],
                                w_sb[:, ko, P * half:P * (half + 1)],
                                xtk[ko][:, jq * QC:(jq + 1) * QC],
                                start=(ko == 0),
                                stop=(ko == KO - 1),
                            )
                        nc.vector.tensor_scalar_add(
                            dst[:, half, jq * QC:(jq + 1) * QC], ps[:],
                            b_sb[:, half:half + 1])

            # ---- V projection: natural layout, scattered into vx ----
            for kb in range(NKB):
                ps = ps_w.tile([P, QC], F32, tag="w", name="ps_v")
                for ko in range(KO):
                    nc.tensor.matmul(
                        ps[:, :DC],
                        xtk[ko][:, kb * P:(kb + 1) * P],
                        wv[:, ko, :],
                        start=(ko == 0),
                        stop=(ko == KO - 1),
                    )
                nc.vector.tensor_add(
                    vx[kb][:, :, 0:HS],
                    ps[:, :DC].rearrange("p (h e) -> p h e", e=HS),
                    bv_sb.rearrange("p (h e) -> p h e", e=HS),
                )

            # ---- output projection for one q-chunk (emitted pipelined) ----
            def emit_wo(j):
                for mo in range(KO):
                    ps = ps_w.tile([P, QC], F32, tag="w", name="ps_wo")
                    for cb in range(2):
                        nc.tensor.matmul(
                            ps[:],
                            wo[:, cb, mo * P:(mo + 1) * P],
                            oc[j][:, cb, :],
                            start=(cb == 0),
                            stop=(cb == 1),
                        )
                    st = stg.tile([P, QC], mm_dt, tag="st", name="st", bufs=4)
                    nc.any.tensor_copy(st[:], ps[:])
                    [nc.sync, nc.gpsimd][mo % 2].dma_start(
                        out_d[:, mo, j * QC:(j + 1) * QC], st[:])

            # ---- attention: q-chunk outer (short chunk last -> short
            #      tail), head-pair (partition half) inner
            seq = [1, 2, 3, 0]
            for si, j in enumerate(seq):
                nkb = 4 * j + 4  # causal: only k blocks 0..4j+3
                deferred = []
                for hp in range(2):
                    pts = {}
                    for kb in range(nkb):
                        # one 2-bank psum tile holds BOTH heads' scores; the
                        # two K=64 matmuls become ready together, schedule
                        # adjacently, and run concurrently in row groups
                        # (0,0)/(64,0); one exp covers both.
                        ps = ps_s.tile([P, 2 * QC], F32, tag="s", name="ps_sc")
                        for hh in range(2):
                            base = HS * hh
                            nc.tensor.matmul(
                                ps[:, hh * QC:(hh + 1) * QC],
                                kt[base:base + HS, hp, kb * P:(kb + 1) * P],
                                qt[base:base + HS, hp, j * QC:(j + 1) * QC],
                                start=True, stop=True,
                            )
                        pt = ptp.tile([P, 2 * QC], mm_dt, tag="pt", name="pt")
                        nc.scalar.activation(pt[:], ps[:], EXP, scale=0.125)
                        if kb >= 4 * j:
                            # zero strictly-above-diagonal: keep q >= k,
                            # i.e. (j*QC + f) - (kb*P + p) >= 0
                            for hh in range(2):
                                nc.gpsimd.affine_select(
                                    pt[:, hh * QC:(hh + 1) * QC],
                                    pt[:, hh * QC:(hh + 1) * QC],
                                    pattern=[[1, QC]],
                                    compare_op=mybir.AluOpType.is_ge,
                                    fill=0.0,
                                    base=j * QC - kb * P,
                                    channel_multiplier=-1,
                                )
                        pts[kb] = pt
                    for hh in range(2):
                        h = 2 * hp + hh
                        po = ps_o.tile([HS + 1, QC], F32, tag="o", name="po")
                        for kb in range(nkb):
                            nc.tensor.matmul(
                                po[:],
                                vx[kb][:, h, :],
                                pts[kb][:, hh * QC:(hh + 1) * QC],
                                start=(kb == 0),
                                stop=(kb == nkb - 1),
                            )
                        oth = otp.tile([HS + 1, QC], F32, tag="ot", name="ot")
                        # ACT copy: frees the PV psum bank without queueing
                        # behind DVE's normalization work
                        nc.scalar.copy(oth[:], po[:])
                        deferred.append((hp, hh, oth))
                # normalization chains after both head-pairs, so the GpSimd
                # broadcasts never queue ahead of the next pair's masks on the
                # in-order GpSimd engine
                for hp, hh, oth in deferred:
                    # bounce denominator row to partition 0 (aligned source
                    # for the broadcast), then reciprocal there
                    rb = stg.tile([1, QC], F32, tag="rb", name="rb")
                    nc.sync.dma_start(rb[:], oth[HS:HS + 1, :])
                    rc = stg.tile([1, QC], F32, tag="rc", name="rc")
                    nc.vector.reciprocal_approx_fast(rc[:], rb[:])
                    bc = stg.tile([HS, QC], F32, tag="bc", name="bc")
                    nc.gpsimd.partition_broadcast(bc[:], rc[0:1, :])
                    on = stg.tile([HS, QC], mm_dt, tag="on", name="on")
                    nc.vector.tensor_mul(out=on[:], in0=oth[:HS, :],
                                         in1=bc[:])
                    nc.sync.dma_start(oc[j][HS * hh:HS * (hh + 1), hp, :],
                                      on[:])
                if si >= 1:
                    emit_wo(seq[si - 1])
            emit_wo(seq[-1])

    nc.compile()
    return nc


def _np_mm_dtype(mm_dt):
    return {
        mybir.dt.bfloat16: ml_dtypes.bfloat16,
        mybir.dt.float32: np.float32,
        mybir.dt.float32r: np.float32,
    }[mm_dt]


def make_in_maps(q, Wq, bq, Wk, bk, Wv, bv, Wo, mm_dt=mybir.dt.bfloat16):
    """Host-side shard + retile of the full inputs into per-core in_maps."""
    np_dt = _np_mm_dtype(mm_dt)
    in_maps = []
    for c in range(NCORES):
        b, g = divmod(c, 4)
        c0 = DC * g
        X = np.asarray(q[b], dtype=np.float32)            # [S, D]
        xt = np.ascontiguousarray(
            X.T.reshape(KO, P, S).transpose(1, 0, 2)).astype(np_dt)

        def wtile(W):  # [D, DC] -> [P, KO, DC]
            return np.ascontiguousarray(
                np.asarray(W, np.float32).reshape(KO, P, DC)
                .transpose(1, 0, 2)).astype(np_dt)

        wq_t = wtile(Wq[:, c0:c0 + DC])
        wk_t = wtile(Wk[:, c0:c0 + DC])
        wv_t = wtile(Wv[:, c0:c0 + DC])
        wo_t = np.ascontiguousarray(
            np.asarray(Wo[c0:c0 + DC, :], np.float32).reshape(2, P, D)
            .transpose(1, 0, 2)).astype(np_dt)
        bq_t = np.ascontiguousarray(
            np.asarray(bq[c0:c0 + DC], np.float32).reshape(2, P).T)
        bk_t = np.ascontiguousarray(
            np.asarray(bk[c0:c0 + DC], np.float32).reshape(2, P).T)
        bv_t = np.ascontiguousarray(
            np.tile(np.asarray(bv[c0:c0 + DC], np.float32)[None, :], (P, 1)))
        in_maps.append({
            "xt": xt, "wq": wq_t, "wk": wk_t, "wv": wv_t, "wo": wo_t,
            "bq": bq_t, "bk": bk_t, "bvb": bv_t,
        })
    return in_maps


def gather_output(results, bo):
    """Sum per-core partials back into the full [B, S, D] output."""
    full = np.zeros((B, S, D), np.float32)
    for c in range(NCORES):
        o = np.asarray(results[c]["out"]).astype(np.float32)  # [P, KO, S]
        partial_t = o.transpose(1, 0, 2).reshape(D, S)    # [D, S]
        full[c // 4] += partial_t.T
    full += np.asarray(bo, np.float32)[None, None, :]
    return full


_NC_CACHE = {}
LAST_RESULTS = None
MM_DT = mybir.dt.bfloat16


def kernel(q, Wq, bq, Wk, bk, Wv, bv, Wo, bo):
    global LAST_RESULTS
    mm_dt = MM_DT
    if mm_dt not in _NC_CACHE:
        _NC_CACHE[mm_dt] = build_program(mm_dt)
    nc = _NC_CACHE[mm_dt]
    in_maps = make_in_maps(q, Wq, bq, Wk, bk, Wv, bv, Wo, mm_dt)
    res = run_bass_kernel_spmd(nc, in_maps, core_ids=list(range(NCORES)))
    LAST_RESULTS = res
    return gather_output(res.results, bo)
